# revision 1
# baseline (speedup 1.0000x reference)
"""NetVLAD Trainium2 kernel.

x:(32,4096,128) f32, clusters:(64,128), clusters2:(1,64,128) ->
vlad:(32, 8192).

Math (validated against the reference, scale-rel err ~2e-6):
  L = x @ C.T                      [N, K]  per batch
  A = softmax(L, axis=K)           (no max subtraction: |L| <= ~83,
                                    exp stays in fp32 range, A <= 1)
  V = A.T @ [x | 1]                [K, D+1]  (col D = a_sum, free via
                                    ones column appended host-side)
  vlad = V[:, :D] - a_sum^2 * c2   (folded as + a_sum^2 * (-c2))

Sharding: data-parallel over batch, 4 batches per core x 8 cores.
Per core: 32 groups of 512 rows (4 chunks of 128).
"""

import os
import sys

import numpy as np

for _p in ("/opt/trn_rl_repo", "/root/.axon_site/_ro/trn_rl_repo"):
    if os.path.isdir(_p) and _p not in sys.path:
        sys.path.insert(0, _p)

import concourse.bass as bass  # noqa: E402
import concourse.tile as tile  # noqa: E402
from concourse import bacc, mybir  # noqa: E402
from concourse.bass_utils import run_bass_kernel_spmd  # noqa: E402

F32 = mybir.dt.float32
F32R = mybir.dt.float32r
NCORES = 8
B_FULL, N, D, K = 32, 4096, 128, 64
BPC = B_FULL // NCORES  # batches per core
P = 128  # rows per chunk
CPG = 4  # chunks per group
NG = N // (P * CPG)  # groups per batch

_ABL = set(os.environ.get("KABL", "").split(","))  # ablation expts; unused in prod
_TRACE = False
_LAST_RESULT = None
_CACHE = {}
_REPEAT = 1  # timing instrumentation: unroll the whole body N times


W = 2  # groups loaded per DMA (batched to amortize 625ns hwdge issue)


def _build():
    nc = bacc.Bacc("TRN2", debug=False)
    xs_e = nc.dram_tensor("xs", [BPC, P, NG, CPG, D + 2], F32R, kind="ExternalInput")
    # packed consts: cols [0:P]=identity, [P:P+K]=ct, [P+K:P+K+D]=c2n (rows 0:K)
    cs_e = nc.dram_tensor("cs", [P, P + K + D], F32, kind="ExternalInput")
    y_e = nc.dram_tensor("y", [K, BPC, D], F32, kind="ExternalOutput")

    with tile.TileContext(nc) as tc:
        with (
            tc.tile_pool(name="consts", bufs=1) as cpool,
            tc.tile_pool(name="idp", bufs=1) as idpool,
            tc.tile_pool(name="xg", bufs=4) as xpool,
            tc.tile_pool(name="xts", bufs=4) as xtpool,
            tc.tile_pool(name="ea", bufs=8) as eapool,
            tc.tile_pool(name="small", bufs=4) as spool,
            tc.tile_pool(name="ob", bufs=2) as opool,
            tc.tile_pool(name="pt", bufs=3, space="PSUM") as ptpool,
            tc.tile_pool(name="pl", bufs=3, space="PSUM") as plpool,
            tc.tile_pool(name="pv", bufs=2, space="PSUM") as pvpool,
        ):
            cs = cpool.tile([P, P + K + D], F32, tag="cs")
            id_s = cs[:, 0:P]
            ct_s = cs[:, P : P + K]
            c2n_s = cs[0:K, P + K : P + K + D]
            ob_all = opool.tile([K, BPC, D], F32, tag="ob")
            dum = opool.tile([1, 1], F32, tag="dum")
            # touch ACT first so its 1.3us LoadActFuncSet overlaps the DMA wait
            nc.vector.memset(dum[:], 0.0)
            nc.scalar.copy(dum[:], dum[:])
            # walrus requires the f32r matmul's stationary operand (identity
            # for transposes) to come from a compute-engine producer, not DMA
            id2 = idpool.tile([P, P], F32R, tag="id2")

            work = [
                (b, g)
                for _ in range(_REPEAT)
                for b in range(BPC)
                for g in range(NG)
            ]
            n = len(work)
            # software-pipeline: iteration i emits
            #   A(i):   dma prefetch, transp(i) [PE], copies(i) [Pool+ACT]
            #   B(i-3): mm2(i-3) [PE] (+ epilogue at batch end)
            #   M(i-1): mm1(i-1) [PE]; exp(i-1) [ACT]; softmax(i-1) [DVE]
            # so mm2's ag dep is 2 iterations old, mm1's xts 1 iteration.
            st = {}
            vp_by_i = {}
            xgw = None
            for i in range(n + 3):
                if i < n:
                    b, g = work[i]
                    if g == 0:
                        vp_new = pvpool.tile([K, 2, D + 2], F32, tag="vp")
                        vp_by_i[i] = vp_new
                    else:
                        vp_by_i[i] = vp_by_i[i - 1]
                    if i == 0:
                        # startup: HWDGE issues serialize at 625ns each, so
                        # order = xg0 (first compute dep), id (transpose dep),
                        # xg1, ct+c2n (mm1 dep, needed one iteration later)
                        xgw = xpool.tile([P, W, CPG, D + 4], F32R, tag="xg")
                        nc.sync.dma_start(
                            xgw[:, 0:1, :, 0 : D + 2], xs_e[b, :, 0:1]
                        )
                        nc.sync.dma_start(cs[:, 0:P], cs_e[:, 0:P])
                        nc.sync.dma_start(
                            xgw[:, 1:2, :, 0 : D + 2], xs_e[b, :, 1:2]
                        )
                        nc.sync.dma_start(cs[:, P:], cs_e[:, P:])
                        nc.gpsimd.tensor_copy(id2[:], id_s)
                    elif g % W == 0:
                        xgw = xpool.tile([P, W, CPG, D + 4], F32R, tag="xg")
                        nc.sync.dma_start(
                            xgw[:, :, :, 0 : D + 2], xs_e[b, :, g : g + W]
                        )
                    xg = xgw[:, g % W]

                    xtp = ptpool.tile([P, CPG, P], F32, tag="xtp")
                    for c in range(CPG):
                        nc.tensor.transpose(
                            xtp[:, c, :].bitcast(F32R), xg[:, c, 0:D], id2[:]
                        )
                    xts = xtpool.tile([P, CPG, P], F32, tag="xts")
                    nc.scalar.copy(xts[:, 0:2, :], xtp[:, 0:2, :])
                    nc.scalar.copy(xts[:, 2:4, :], xtp[:, 2:4, :])
                    st[i] = [b, g, xg, xts, None]

                if 0 <= i - 3 < n:
                    bb, gg, xgB, _, agB = st.pop(i - 3)
                    vpB = vp_by_i.pop(i - 3)
                    for c in range(CPG):
                        # f32r with out free >= 256 runs at 1 cyc/row (vs 4
                        # for fp32); duplicate the rhs via a stride-0 repeat
                        # so out free = 2*(D+2) = 260 (D+2: fp32r ISA needs even
                        # innermost extents; col D+1 is a zero pad).
                        rhs = (
                            xgB[:, c, 0 : D + 2]
                            .unsqueeze(1)
                            .broadcast_to([P, 2, D + 2])
                        )
                        nc.tensor.matmul(
                            vpB[:],
                            agB[:, c, :],
                            rhs,
                            start=(gg == 0 and c == 0),
                            stop=(gg == NG - 1 and c == CPG - 1),
                        )
                    if gg == NG - 1:
                        asq = spool.tile([K, 1], F32, tag="asq")
                        nc.scalar.square(asq[:], vpB[:, 0, D : D + 1])
                        nc.vector.scalar_tensor_tensor(
                            ob_all[:, bb, :],
                            c2n_s,
                            asq[:],
                            vpB[:, 0, 0:D],
                            mybir.AluOpType.mult,
                            mybir.AluOpType.add,
                        )
                        if i - 3 == n - 1:
                            nc.sync.dma_start(y_e[:], ob_all[:])

                if 0 <= i - 1 < n:
                    sM = st[i - 1]
                    xtsM = sM[3]
                    lp = plpool.tile([P, CPG, K], F32, tag="lp")
                    for c in range(CPG):
                        nc.tensor.matmul(
                            lp[:, c, :], xtsM[:, c, :], ct_s, start=True, stop=True
                        )
                    eg = eapool.tile([P, CPG, K], F32, tag="eg")
                    nc.scalar.activation(eg[:], lp[:], mybir.ActivationFunctionType.Exp)
                    sg = spool.tile([P, CPG], F32, tag="sg")
                    nc.vector.tensor_reduce(
                        sg[:], eg[:], mybir.AxisListType.X, mybir.AluOpType.add
                    )
                    rg = spool.tile([P, CPG], F32, tag="rg")
                    nc.vector.reciprocal(rg[:], sg[:])
                    ag = eapool.tile([P, CPG, K], F32R, tag="ag")
                    for c in range(CPG):
                        nc.vector.tensor_scalar_mul(
                            ag[:, c, :], eg[:, c, :].bitcast(F32R), rg[:, c : c + 1]
                        )
                    sM[4] = ag

    nc.compile()
    return nc


def _prep_inputs(x, clusters, clusters2):
    x = np.asarray(x, np.float32)
    ct = np.asarray(clusters, np.float32).T  # [D, K]
    c2n = -np.asarray(clusters2, np.float32)[0]  # [K, D]
    cs = np.zeros((P, P + K + D), np.float32)
    cs[:, 0:P] = np.eye(P, dtype=np.float32)
    cs[:, P : P + K] = ct
    cs[0:K, P + K : P + K + D] = c2n
    # [core, b, g, c, p, d] -> [core, b, p, g, c, d]; append ones col (a_sum
    # via mm2) then a zero pad col (fp32r ISA wants even innermost extents)
    xr = x.reshape(NCORES, BPC, NG, CPG, P, D).transpose(0, 1, 4, 2, 3, 5)
    pad = np.zeros((NCORES, BPC, P, NG, CPG, 2), np.float32)
    pad[..., 0] = 1.0
    xs = np.ascontiguousarray(np.concatenate([xr, pad], axis=-1))
    return [{"xs": xs[i], "cs": cs} for i in range(NCORES)]


def kernel(x, clusters, clusters2):
    global _LAST_RESULT
    if "nc" not in _CACHE:
        _CACHE["nc"] = _build()
    nc = _CACHE["nc"]
    in_maps = _prep_inputs(x, clusters, clusters2)
    res = run_bass_kernel_spmd(nc, in_maps, list(range(NCORES)), trace=_TRACE)
    _LAST_RESULT = res
    # per-core y is [K, BPC, D] -> [BPC, K, D]
    y = np.stack([np.asarray(res.results[i]["y"]) for i in range(NCORES)])
    return y.transpose(0, 2, 1, 3).reshape(B_FULL, K * D).astype(np.float32)



# revision 4
# speedup vs baseline: 3.2867x; 3.2867x over previous
"""NetVLAD Trainium2 kernel (v2: fp16 wire format, native input layout).

x:(32,4096,128) f32, clusters:(64,128), clusters2:(1,64,128) ->
vlad:(32, 8192).

Math (validated against the reference; fp16 pipeline rel err ~3e-4,
tolerance 2e-2):
  L = x @ C.T                      [N, K]  per batch (f32 PSUM)
  A = softmax(L, axis=K)           (no max subtraction: |L| <= ~83,
                                    exp stays in fp32 range, A <= 1)
  V = A.T @ [x | 1]                [K, D+1]  (col D = a_sum via a ones
                                    column kept in SBUF, memset once)
  vlad = V[:, :D] - a_sum^2 * c2   (folded as + a_sum^2 * (-c2))

Wall-clock is dominated by the axon tunnel (~90 MB/s H2D) and per-call
dispatch, so inputs ship as fp16 (halves bytes) in the native x layout
(host prep is a single astype; the DMA access pattern does the
chunk-major permute on device). All constants ride in one tensor to
minimize per-shard RPC count.

Sharding: data-parallel over batch, 4 batches per core x 8 cores.
Per core: 8 groups/batch of 512 rows (4 chunks of 128).
"""

import os
import sys

import numpy as np

for _p in ("/opt/trn_rl_repo", "/root/.axon_site/_ro/trn_rl_repo"):
    if os.path.isdir(_p) and _p not in sys.path:
        sys.path.insert(0, _p)

import concourse.bass as bass  # noqa: E402
import concourse.tile as tile  # noqa: E402
from concourse import bacc, mybir  # noqa: E402
from concourse.bass_utils import run_bass_kernel_spmd  # noqa: E402

F32 = mybir.dt.float32
F16 = mybir.dt.float16
NCORES = 8
B_FULL, N, D, K = 32, 4096, 128, 64
BPC = B_FULL // NCORES  # batches per core
P = 128  # rows per chunk
CPG = 4  # chunks per group
NG = N // (P * CPG)  # groups per batch
NCH = N // P  # chunks per batch
W = 2  # groups loaded per DMA
WC = W * CPG  # chunks per DMA
NBUF = 4  # x-tile ring buffers

_TRACE = False
_LAST_RESULT = None
_CACHE = {}


def _build():
    nc = bacc.Bacc("TRN2", debug=False)
    xs_e = nc.dram_tensor("xs", [BPC, NCH, P, D], F16, kind="ExternalInput")
    # packed consts: cols [0:P]=identity, [P:P+K]=ct, [P+K:P+K+D]=c2n (rows 0:K)
    cs_e = nc.dram_tensor("cs", [P, P + K + D], F16, kind="ExternalInput")
    y_e = nc.dram_tensor("y", [K, BPC, D], F32, kind="ExternalOutput")

    with tile.TileContext(nc) as tc:
        with (
            tc.tile_pool(name="consts", bufs=1) as cpool,
            tc.tile_pool(name="idp", bufs=1) as idpool,
            tc.tile_pool(name="c2p", bufs=1) as c2pool,
            tc.tile_pool(name="xw", bufs=NBUF) as xpool,
            tc.tile_pool(name="xts", bufs=4) as xtpool,
            tc.tile_pool(name="ea", bufs=8) as eapool,
            tc.tile_pool(name="small", bufs=4) as spool,
            tc.tile_pool(name="ob", bufs=2) as opool,
            tc.tile_pool(name="pt", bufs=3, space="PSUM") as ptpool,
            tc.tile_pool(name="pl", bufs=3, space="PSUM") as plpool,
            tc.tile_pool(name="pv", bufs=2, space="PSUM") as pvpool,
        ):
            cs = cpool.tile([P, P + K + D], F16, tag="cs")
            id_s = cs[:, 0:P]
            ct_s = cs[:, P : P + K]
            ob_all = opool.tile([K, BPC, D], F32, tag="ob")
            dum = opool.tile([1, 1], F32, tag="dum")
            # touch ACT first so its 1.3us LoadActFuncSet overlaps the DMA wait
            nc.vector.memset(dum[:], 0.0)
            nc.scalar.copy(dum[:], dum[:])
            # walrus requires the transpose's identity operand to come from a
            # compute-engine producer, not DMA
            id2 = idpool.tile([P, P], F16, tag="id2")
            # c2n upconverted to f32 once so the epilogue STT runs all-f32
            c2f = c2pool.tile([K, D], F32, tag="c2f")
            # x ring buffers; col D = 1.0 (a_sum via mm2), col D+1 = 0 pad.
            # DMA only ever writes cols 0:D, so the memset survives reuse.
            xws = [
                xpool.tile([P, WC, D + 2], F16, name=f"xw{j}", tag=f"xw{j}")
                for j in range(NBUF)
            ]
            for xw in xws:
                nc.vector.memset(xw[:, :, D : D + 1], 1.0)
                nc.vector.memset(xw[:, :, D + 1 : D + 2], 0.0)

            work = [(b, g) for b in range(BPC) for g in range(NG)]
            n = len(work)
            # software-pipeline: iteration i emits
            #   A(i):   dma prefetch, transp(i) [PE], copies(i) [ACT]
            #   B(i-3): mm2(i-3) [PE] (+ epilogue at batch end)
            #   M(i-1): mm1(i-1) [PE]; exp(i-1) [ACT]; softmax(i-1) [DVE]
            st = {}
            vp_by_i = {}
            xw_cur = None
            for i in range(n + 3):
                if i < n:
                    b, g = work[i]
                    if g == 0:
                        vp_new = pvpool.tile([K, D + 2], F32, tag="vp")
                        vp_by_i[i] = vp_new
                    else:
                        vp_by_i[i] = vp_by_i[i - 1]
                    if g % W == 0:
                        xw_cur = xws[((b * NG + g) // W) % NBUF]
                        src = xs_e[b, g * CPG : g * CPG + WC].transpose([1, 0, 2])
                        nc.sync.dma_start(xw_cur[:, :, 0:D], src)
                        if i == 0:
                            # startup: consts after the first x block so the
                            # first compute dep is in flight first
                            nc.sync.dma_start(cs[:], cs_e[:])
                            nc.gpsimd.tensor_copy(id2[:], id_s)
                            nc.scalar.copy(c2f[:], cs[0:K, P + K : P + K + D])
                    cb = (g % W) * CPG
                    xg = xw_cur[:, cb : cb + CPG]

                    xtp = ptpool.tile([P, CPG, P], F16, tag="xtp")
                    for c in range(CPG):
                        nc.tensor.transpose(xtp[:, c, :], xg[:, c, 0:D], id2[:])
                    xts = xtpool.tile([P, CPG, P], F16, tag="xts")
                    nc.scalar.copy(xts[:, 0:2, :], xtp[:, 0:2, :])
                    nc.scalar.copy(xts[:, 2:4, :], xtp[:, 2:4, :])
                    st[i] = [b, g, xg, xts, None]

                if 0 <= i - 3 < n:
                    bb, gg, xgB, _, agB = st.pop(i - 3)
                    vpB = vp_by_i.pop(i - 3)
                    for c in range(CPG):
                        nc.tensor.matmul(
                            vpB[:],
                            agB[:, c, :],
                            xgB[:, c, :],
                            start=(gg == 0 and c == 0),
                            stop=(gg == NG - 1 and c == CPG - 1),
                        )
                    if gg == NG - 1:
                        asq = spool.tile([K, 1], F32, tag="asq")
                        nc.scalar.square(asq[:], vpB[:, D : D + 1])
                        nc.vector.scalar_tensor_tensor(
                            ob_all[:, bb, :],
                            c2f[:],
                            asq[:],
                            vpB[:, 0:D],
                            mybir.AluOpType.mult,
                            mybir.AluOpType.add,
                        )
                        if i - 3 == n - 1:
                            nc.sync.dma_start(y_e[:], ob_all[:])

                if 0 <= i - 1 < n:
                    sM = st[i - 1]
                    xtsM = sM[3]
                    lp = plpool.tile([P, CPG, K], F32, tag="lp")
                    for c in range(CPG):
                        nc.tensor.matmul(
                            lp[:, c, :], xtsM[:, c, :], ct_s, start=True, stop=True
                        )
                    eg = eapool.tile([P, CPG, K], F32, tag="eg")
                    nc.scalar.activation(eg[:], lp[:], mybir.ActivationFunctionType.Exp)
                    sg = spool.tile([P, CPG], F32, tag="sg")
                    nc.vector.tensor_reduce(
                        sg[:], eg[:], mybir.AxisListType.X, mybir.AluOpType.add
                    )
                    rg = spool.tile([P, CPG], F32, tag="rg")
                    nc.vector.reciprocal(rg[:], sg[:])
                    ag = eapool.tile([P, CPG, K], F16, tag="ag")
                    for c in range(CPG):
                        nc.vector.tensor_scalar_mul(
                            ag[:, c, :], eg[:, c, :], rg[:, c : c + 1]
                        )
                    sM[4] = ag

    nc.compile()
    return nc


def _prep_inputs(x, clusters, clusters2):
    xh = np.asarray(x, np.float32).astype(np.float16)
    ct = np.asarray(clusters, np.float32).T.astype(np.float16)  # [D, K]
    c2n = (-np.asarray(clusters2, np.float32)[0]).astype(np.float16)  # [K, D]
    cs = np.zeros((P, P + K + D), np.float16)
    cs[:, 0:P] = np.eye(P, dtype=np.float16)
    cs[:, P : P + K] = ct
    cs[0:K, P + K :] = c2n
    xs = xh.reshape(NCORES, BPC, NCH, P, D)  # pure view of the astype result
    return [{"xs": xs[i], "cs": cs} for i in range(NCORES)]


def kernel(x, clusters, clusters2):
    global _LAST_RESULT
    if "nc" not in _CACHE:
        _CACHE["nc"] = _build()
    nc = _CACHE["nc"]
    in_maps = _prep_inputs(x, clusters, clusters2)
    res = run_bass_kernel_spmd(nc, in_maps, list(range(NCORES)), trace=_TRACE)
    _LAST_RESULT = res
    # per-core y is [K, BPC, D] -> [BPC, K, D]
    y = np.stack([np.asarray(res.results[i]["y"]) for i in range(NCORES)])
    return y.transpose(0, 2, 1, 3).reshape(B_FULL, K * D).astype(np.float32)


# revision 5
# speedup vs baseline: 3.5974x; 1.0945x over previous
"""NetVLAD Trainium2 kernel (v3: fp16 wire format, native layout, single
input tensor, persistent jit cache).

x:(32,4096,128) f32, clusters:(64,128), clusters2:(1,64,128) ->
vlad:(32, 8192).

Math (validated against the reference; fp16 pipeline rel err ~3.7e-4,
tolerance 2e-2):
  L = x @ C.T                      [N, K]  per batch (f32 PSUM)
  A = softmax(L, axis=K)           (no max subtraction: |L| <= ~83,
                                    exp stays in fp32 range, A <= 1)
  V = A.T @ [x | 1]                [K, D+1]  (col D = a_sum via a ones
                                    column kept in SBUF, memset once)
  vlad = V[:, :D] - a_sum^2 * c2   (folded as + a_sum^2 * (-c2))

Wall-clock is dominated by the axon tunnel (~90 MB/s H2D) and per-call
fixed costs (fresh jax.jit + XLA compile inside run_bass_kernel_spmd,
~15ms per tensor put), so:
  - inputs ship as fp16 (halves bytes) in the native x layout; host prep
    is one parallel f32->f16 cast (no transpose) into a cached buffer,
    and the DMA access pattern does the chunk-major permute on device
  - identity/ct/c2n constants ride as 3 extra chunks of the same tensor
    (one put instead of two per shard)
  - the jax persistent compilation cache turns the per-call XLA compile
    into a ~30ms disk hit and avoids the fresh-executable init penalty

Sharding: data-parallel over batch, 4 batches per core x 8 cores.
Per core: 8 groups/batch of 512 rows (4 chunks of 128).
"""

import os
import sys
from concurrent.futures import ThreadPoolExecutor

import numpy as np

for _p in ("/opt/trn_rl_repo", "/root/.axon_site/_ro/trn_rl_repo"):
    if os.path.isdir(_p) and _p not in sys.path:
        sys.path.insert(0, _p)

import concourse.bass as bass  # noqa: E402
import concourse.tile as tile  # noqa: E402
from concourse import bacc, mybir  # noqa: E402
from concourse.bass_utils import run_bass_kernel_spmd  # noqa: E402

F32 = mybir.dt.float32
F16 = mybir.dt.float16
NCORES = 8
B_FULL, N, D, K = 32, 4096, 128, 64
BPC = B_FULL // NCORES  # batches per core
P = 128  # rows per chunk
CPG = 4  # chunks per group
NG = N // (P * CPG)  # groups per batch
NCH = N // P  # chunks per batch
NCHT = BPC * NCH  # x chunks per core; consts live at chunks NCHT..NCHT+2
W = 2  # groups loaded per DMA
WC = W * CPG  # chunks per DMA
NBUF = 4  # x-tile ring buffers

_TRACE = False
_LAST_RESULT = None
_CACHE = {}
_POOL = ThreadPoolExecutor(NCORES)


def _build():
    nc = bacc.Bacc("TRN2", debug=False)
    # chunks 0..NCHT-1: x data (batch-major); chunk NCHT: identity,
    # NCHT+1 cols 0:K: ct, NCHT+2 rows 0:K: c2n
    xs_e = nc.dram_tensor("xs", [NCHT + 3, P, D], F16, kind="ExternalInput")
    y_e = nc.dram_tensor("y", [K, BPC, D], F32, kind="ExternalOutput")

    with tile.TileContext(nc) as tc:
        with (
            tc.tile_pool(name="consts", bufs=1) as cpool,
            tc.tile_pool(name="idp", bufs=1) as idpool,
            tc.tile_pool(name="c2p", bufs=1) as c2pool,
            tc.tile_pool(name="xw", bufs=NBUF) as xpool,
            tc.tile_pool(name="xts", bufs=4) as xtpool,
            tc.tile_pool(name="ea", bufs=8) as eapool,
            tc.tile_pool(name="small", bufs=4) as spool,
            tc.tile_pool(name="ob", bufs=2) as opool,
            tc.tile_pool(name="pt", bufs=3, space="PSUM") as ptpool,
            tc.tile_pool(name="pl", bufs=3, space="PSUM") as plpool,
            tc.tile_pool(name="pv", bufs=2, space="PSUM") as pvpool,
        ):
            cs = cpool.tile([P, 3, D], F16, tag="cs")
            id_s = cs[:, 0, :]
            ct_s = cs[:, 1, 0:K]
            ob_all = opool.tile([K, BPC, D], F32, tag="ob")
            dum = opool.tile([1, 1], F32, tag="dum")
            # touch ACT first so its 1.3us LoadActFuncSet overlaps the DMA wait
            nc.vector.memset(dum[:], 0.0)
            nc.scalar.copy(dum[:], dum[:])
            # walrus requires the transpose's identity operand to come from a
            # compute-engine producer, not DMA
            id2 = idpool.tile([P, P], F16, tag="id2")
            # c2n upconverted to f32 once so the epilogue STT runs all-f32
            c2f = c2pool.tile([K, D], F32, tag="c2f")
            # x ring buffers; col D = 1.0 (a_sum via mm2), col D+1 = 0 pad.
            # DMA only ever writes cols 0:D, so the memset survives reuse.
            xws = [
                xpool.tile([P, WC, D + 2], F16, name=f"xw{j}", tag=f"xw{j}")
                for j in range(NBUF)
            ]
            for xw in xws:
                nc.vector.memset(xw[:, :, D : D + 1], 1.0)
                nc.vector.memset(xw[:, :, D + 1 : D + 2], 0.0)

            work = [(b, g) for b in range(BPC) for g in range(NG)]
            n = len(work)
            # software-pipeline: iteration i emits
            #   A(i):   dma prefetch, transp(i) [PE], copies(i) [ACT]
            #   B(i-3): mm2(i-3) [PE] (+ epilogue at batch end)
            #   M(i-1): mm1(i-1) [PE]; exp(i-1) [ACT]; softmax(i-1) [DVE]
            st = {}
            vp_by_i = {}
            xw_cur = None
            for i in range(n + 3):
                if i < n:
                    b, g = work[i]
                    if g == 0:
                        vp_new = pvpool.tile([K, D + 2], F32, tag="vp")
                        vp_by_i[i] = vp_new
                    else:
                        vp_by_i[i] = vp_by_i[i - 1]
                    if g % W == 0:
                        xw_cur = xws[((b * NG + g) // W) % NBUF]
                        cb0 = b * NCH + g * CPG
                        src = xs_e[cb0 : cb0 + WC].transpose([1, 0, 2])
                        nc.sync.dma_start(xw_cur[:, :, 0:D], src)
                        if i == 0:
                            # startup: consts after the first x block so the
                            # first compute dep is in flight first
                            nc.sync.dma_start(
                                cs[:], xs_e[NCHT : NCHT + 3].transpose([1, 0, 2])
                            )
                            nc.gpsimd.tensor_copy(id2[:], id_s)
                            nc.scalar.copy(c2f[:], cs[0:K, 2, :])
                    cb = (g % W) * CPG
                    xg = xw_cur[:, cb : cb + CPG]

                    xtp = ptpool.tile([P, CPG, P], F16, tag="xtp")
                    for c in range(CPG):
                        nc.tensor.transpose(xtp[:, c, :], xg[:, c, 0:D], id2[:])
                    xts = xtpool.tile([P, CPG, P], F16, tag="xts")
                    nc.scalar.copy(xts[:, 0:2, :], xtp[:, 0:2, :])
                    nc.scalar.copy(xts[:, 2:4, :], xtp[:, 2:4, :])
                    st[i] = [b, g, xg, xts, None]

                if 0 <= i - 3 < n:
                    bb, gg, xgB, _, agB = st.pop(i - 3)
                    vpB = vp_by_i.pop(i - 3)
                    for c in range(CPG):
                        nc.tensor.matmul(
                            vpB[:],
                            agB[:, c, :],
                            xgB[:, c, :],
                            start=(gg == 0 and c == 0),
                            stop=(gg == NG - 1 and c == CPG - 1),
                        )
                    if gg == NG - 1:
                        asq = spool.tile([K, 1], F32, tag="asq")
                        nc.scalar.square(asq[:], vpB[:, D : D + 1])
                        nc.vector.scalar_tensor_tensor(
                            ob_all[:, bb, :],
                            c2f[:],
                            asq[:],
                            vpB[:, 0:D],
                            mybir.AluOpType.mult,
                            mybir.AluOpType.add,
                        )
                        if i - 3 == n - 1:
                            nc.sync.dma_start(y_e[:], ob_all[:])

                if 0 <= i - 1 < n:
                    sM = st[i - 1]
                    xtsM = sM[3]
                    lp = plpool.tile([P, CPG, K], F32, tag="lp")
                    for c in range(CPG):
                        nc.tensor.matmul(
                            lp[:, c, :], xtsM[:, c, :], ct_s, start=True, stop=True
                        )
                    eg = eapool.tile([P, CPG, K], F32, tag="eg")
                    nc.scalar.activation(eg[:], lp[:], mybir.ActivationFunctionType.Exp)
                    sg = spool.tile([P, CPG], F32, tag="sg")
                    nc.vector.tensor_reduce(
                        sg[:], eg[:], mybir.AxisListType.X, mybir.AluOpType.add
                    )
                    rg = spool.tile([P, CPG], F32, tag="rg")
                    nc.vector.reciprocal(rg[:], sg[:])
                    ag = eapool.tile([P, CPG, K], F16, tag="ag")
                    for c in range(CPG):
                        nc.vector.tensor_scalar_mul(
                            ag[:, c, :], eg[:, c, :], rg[:, c : c + 1]
                        )
                    sM[4] = ag

    nc.compile()
    return nc


def _enable_jax_cache():
    try:
        import jax

        jax.config.update("jax_compilation_cache_dir", "/tmp/jax_bass_cache")
        jax.config.update("jax_persistent_cache_min_compile_time_secs", 0.0)
        jax.config.update("jax_persistent_cache_min_entry_size_bytes", 0)
    except Exception:
        pass


def _host_buffer():
    if "hb" not in _CACHE:
        hb = np.zeros((NCORES, NCHT + 3, P, D), np.float16)
        hb[:, NCHT] = np.eye(P, dtype=np.float16)
        _CACHE["hb"] = hb
    return _CACHE["hb"]


def _prep_inputs(x, clusters, clusters2):
    hb = _host_buffer()
    xr = np.asarray(x, np.float32).reshape(NCORES, NCHT, P, D)
    futs = [
        _POOL.submit(np.copyto, hb[i, 0:NCHT], xr[i], casting="same_kind")
        for i in range(NCORES)
    ]
    ct = np.asarray(clusters, np.float32).T.astype(np.float16)  # [D, K]
    c2n = (-np.asarray(clusters2, np.float32)[0]).astype(np.float16)  # [K, D]
    hb[:, NCHT + 1, :, 0:K] = ct
    hb[:, NCHT + 2, 0:K, :] = c2n
    for f in futs:
        f.result()
    return [{"xs": hb[i]} for i in range(NCORES)]


def kernel(x, clusters, clusters2):
    global _LAST_RESULT
    _enable_jax_cache()
    if "nc" not in _CACHE:
        _CACHE["nc"] = _build()
    nc = _CACHE["nc"]
    in_maps = _prep_inputs(x, clusters, clusters2)
    res = run_bass_kernel_spmd(nc, in_maps, list(range(NCORES)), trace=_TRACE)
    _LAST_RESULT = res
    # per-core y is [K, BPC, D] -> [BPC, K, D]
    y = np.stack([np.asarray(res.results[i]["y"]) for i in range(NCORES)])
    return y.transpose(0, 2, 1, 3).reshape(B_FULL, K * D).astype(np.float32)


# revision 6
# speedup vs baseline: 4.0973x; 1.1390x over previous
"""NetVLAD Trainium2 kernel (v3: fp16 wire format, native layout, single
input tensor, persistent jit cache).

x:(32,4096,128) f32, clusters:(64,128), clusters2:(1,64,128) ->
vlad:(32, 8192).

Math (validated against the reference; fp16 pipeline rel err ~3.7e-4,
tolerance 2e-2):
  L = x @ C.T                      [N, K]  per batch (f32 PSUM)
  A = softmax(L, axis=K)           (no max subtraction: |L| <= ~83,
                                    exp stays in fp32 range, A <= 1)
  V = A.T @ [x | 1]                [K, D+1]  (col D = a_sum via a ones
                                    column kept in SBUF, memset once)
  vlad = V[:, :D] - a_sum^2 * c2   (folded as + a_sum^2 * (-c2))

Wall-clock is dominated by the axon tunnel (~90 MB/s H2D) and per-call
fixed costs (fresh jax.jit + XLA compile inside run_bass_kernel_spmd,
~15ms per tensor put), so:
  - inputs ship as fp16 (halves bytes) in the native x layout; host prep
    is one parallel f32->f16 cast (no transpose) into a cached buffer,
    and the DMA access pattern does the chunk-major permute on device
  - identity/ct/c2n constants ride as 3 extra chunks of the same tensor
    (one put instead of two per shard)
  - the jax persistent compilation cache turns the per-call XLA compile
    into a ~30ms disk hit and avoids the fresh-executable init penalty

Sharding: data-parallel over batch, 4 batches per core x 8 cores.
Per core: 8 groups/batch of 512 rows (4 chunks of 128).
"""

import os
import sys
from concurrent.futures import ThreadPoolExecutor

import numpy as np

for _p in ("/opt/trn_rl_repo", "/root/.axon_site/_ro/trn_rl_repo"):
    if os.path.isdir(_p) and _p not in sys.path:
        sys.path.insert(0, _p)

import concourse.bass as bass  # noqa: E402
import concourse.tile as tile  # noqa: E402
from concourse import bacc, mybir  # noqa: E402
from concourse.bass_utils import run_bass_kernel_spmd  # noqa: E402

F32 = mybir.dt.float32
F16 = mybir.dt.float16
NCORES = 8
B_FULL, N, D, K = 32, 4096, 128, 64
BPC = B_FULL // NCORES  # batches per core
P = 128  # rows per chunk
CPG = 4  # chunks per group
NG = N // (P * CPG)  # groups per batch
NCH = N // P  # chunks per batch
NCHT = BPC * NCH  # x chunks per core; consts live at chunks NCHT..NCHT+2
W = 2  # groups loaded per DMA
WC = W * CPG  # chunks per DMA
NBUF = 4  # x-tile ring buffers

_TRACE = False
_LAST_RESULT = None
_CACHE = {}
_POOL = ThreadPoolExecutor(NCORES)


def _build():
    nc = bacc.Bacc("TRN2", debug=False)
    # chunks 0..NCHT-1: x data (batch-major); chunk NCHT: identity,
    # NCHT+1 cols 0:K: ct, NCHT+2 rows 0:K: c2n
    xs_e = nc.dram_tensor("xs", [NCHT + 3, P, D], F16, kind="ExternalInput")
    y_e = nc.dram_tensor("y", [K, BPC, D], F16, kind="ExternalOutput")

    with tile.TileContext(nc) as tc:
        with (
            tc.tile_pool(name="consts", bufs=1) as cpool,
            tc.tile_pool(name="idp", bufs=1) as idpool,
            tc.tile_pool(name="c2p", bufs=1) as c2pool,
            tc.tile_pool(name="xw", bufs=NBUF) as xpool,
            tc.tile_pool(name="xts", bufs=4) as xtpool,
            tc.tile_pool(name="ea", bufs=8) as eapool,
            tc.tile_pool(name="small", bufs=4) as spool,
            tc.tile_pool(name="ob", bufs=2) as opool,
            tc.tile_pool(name="pt", bufs=3, space="PSUM") as ptpool,
            tc.tile_pool(name="pl", bufs=3, space="PSUM") as plpool,
            tc.tile_pool(name="pv", bufs=2, space="PSUM") as pvpool,
        ):
            cs = cpool.tile([P, 3, D], F16, tag="cs")
            id_s = cs[:, 0, :]
            ct_s = cs[:, 1, 0:K]
            ob_all = opool.tile([K, BPC, D], F16, tag="ob")
            dum = opool.tile([1, 1], F32, tag="dum")
            # touch ACT first so its 1.3us LoadActFuncSet overlaps the DMA wait
            nc.vector.memset(dum[:], 0.0)
            nc.scalar.copy(dum[:], dum[:])
            # walrus requires the transpose's identity operand to come from a
            # compute-engine producer, not DMA
            id2 = idpool.tile([P, P], F16, tag="id2")
            # c2n upconverted to f32 once so the epilogue STT runs all-f32
            c2f = c2pool.tile([K, D], F32, tag="c2f")
            # x ring buffers; col D = 1.0 (a_sum via mm2), col D+1 = 0 pad.
            # DMA only ever writes cols 0:D, so the memset survives reuse.
            xws = [
                xpool.tile([P, WC, D + 2], F16, name=f"xw{j}", tag=f"xw{j}")
                for j in range(NBUF)
            ]
            for xw in xws:
                nc.vector.memset(xw[:, :, D : D + 1], 1.0)
                nc.vector.memset(xw[:, :, D + 1 : D + 2], 0.0)

            work = [(b, g) for b in range(BPC) for g in range(NG)]
            n = len(work)
            # software-pipeline: iteration i emits
            #   A(i):   dma prefetch, transp(i) [PE], copies(i) [ACT]
            #   B(i-3): mm2(i-3) [PE] (+ epilogue at batch end)
            #   M(i-1): mm1(i-1) [PE]; exp(i-1) [ACT]; softmax(i-1) [DVE]
            st = {}
            vp_by_i = {}
            xw_cur = None
            for i in range(n + 3):
                if i < n:
                    b, g = work[i]
                    if g == 0:
                        vp_new = pvpool.tile([K, D + 2], F32, tag="vp")
                        vp_by_i[i] = vp_new
                    else:
                        vp_by_i[i] = vp_by_i[i - 1]
                    if g % W == 0:
                        xw_cur = xws[((b * NG + g) // W) % NBUF]
                        cb0 = b * NCH + g * CPG
                        src = xs_e[cb0 : cb0 + WC].transpose([1, 0, 2])
                        nc.sync.dma_start(xw_cur[:, :, 0:D], src)
                        if i == 0:
                            # startup: consts after the first x block so the
                            # first compute dep is in flight first
                            nc.sync.dma_start(
                                cs[:], xs_e[NCHT : NCHT + 3].transpose([1, 0, 2])
                            )
                            nc.gpsimd.tensor_copy(id2[:], id_s)
                            nc.scalar.copy(c2f[:], cs[0:K, 2, :])
                    cb = (g % W) * CPG
                    xg = xw_cur[:, cb : cb + CPG]

                    xtp = ptpool.tile([P, CPG, P], F16, tag="xtp")
                    for c in range(CPG):
                        nc.tensor.transpose(xtp[:, c, :], xg[:, c, 0:D], id2[:])
                    xts = xtpool.tile([P, CPG, P], F16, tag="xts")
                    nc.scalar.copy(xts[:, 0:2, :], xtp[:, 0:2, :])
                    nc.scalar.copy(xts[:, 2:4, :], xtp[:, 2:4, :])
                    st[i] = [b, g, xg, xts, None]

                if 0 <= i - 3 < n:
                    bb, gg, xgB, _, agB = st.pop(i - 3)
                    vpB = vp_by_i.pop(i - 3)
                    for c in range(CPG):
                        nc.tensor.matmul(
                            vpB[:],
                            agB[:, c, :],
                            xgB[:, c, :],
                            start=(gg == 0 and c == 0),
                            stop=(gg == NG - 1 and c == CPG - 1),
                        )
                    if gg == NG - 1:
                        asq = spool.tile([K, 1], F32, tag="asq")
                        nc.scalar.square(asq[:], vpB[:, D : D + 1])
                        nc.vector.scalar_tensor_tensor(
                            ob_all[:, bb, :],
                            c2f[:],
                            asq[:],
                            vpB[:, 0:D],
                            mybir.AluOpType.mult,
                            mybir.AluOpType.add,
                        )
                        if i - 3 == n - 1:
                            nc.sync.dma_start(y_e[:], ob_all[:])

                if 0 <= i - 1 < n:
                    sM = st[i - 1]
                    xtsM = sM[3]
                    lp = plpool.tile([P, CPG, K], F32, tag="lp")
                    for c in range(CPG):
                        nc.tensor.matmul(
                            lp[:, c, :], xtsM[:, c, :], ct_s, start=True, stop=True
                        )
                    eg = eapool.tile([P, CPG, K], F32, tag="eg")
                    nc.scalar.activation(eg[:], lp[:], mybir.ActivationFunctionType.Exp)
                    sg = spool.tile([P, CPG], F32, tag="sg")
                    nc.vector.tensor_reduce(
                        sg[:], eg[:], mybir.AxisListType.X, mybir.AluOpType.add
                    )
                    rg = spool.tile([P, CPG], F32, tag="rg")
                    nc.vector.reciprocal(rg[:], sg[:])
                    ag = eapool.tile([P, CPG, K], F16, tag="ag")
                    for c in range(CPG):
                        nc.vector.tensor_scalar(
                            ag[:, c, :],
                            eg[:, c, :],
                            rg[:, c : c + 1],
                            0.25,
                            mybir.AluOpType.mult,
                            mybir.AluOpType.mult,
                        )
                    sM[4] = ag

    nc.compile()
    return nc


def _enable_jax_cache():
    try:
        import jax

        jax.config.update("jax_compilation_cache_dir", "/tmp/jax_bass_cache")
        jax.config.update("jax_persistent_cache_min_compile_time_secs", 0.0)
        jax.config.update("jax_persistent_cache_min_entry_size_bytes", 0)
    except Exception:
        pass


def _host_buffer():
    if "hb" not in _CACHE:
        hb = np.zeros((NCORES, NCHT + 3, P, D), np.float16)
        hb[:, NCHT] = np.eye(P, dtype=np.float16)
        _CACHE["hb"] = hb
    return _CACHE["hb"]


def _prep_inputs(x, clusters, clusters2):
    hb = _host_buffer()
    xr = np.asarray(x, np.float32).reshape(NCORES, NCHT, P, D)
    futs = [
        _POOL.submit(np.copyto, hb[i, 0:NCHT], xr[i], casting="same_kind")
        for i in range(NCORES)
    ]
    ct = np.asarray(clusters, np.float32).T.astype(np.float16)  # [D, K]
    c2n = (-4.0 * np.asarray(clusters2, np.float32)[0]).astype(np.float16)  # [K, D]
    hb[:, NCHT + 1, :, 0:K] = ct
    hb[:, NCHT + 2, 0:K, :] = c2n
    for f in futs:
        f.result()
    return [{"xs": hb[i]} for i in range(NCORES)]


def kernel(x, clusters, clusters2):
    global _LAST_RESULT
    _enable_jax_cache()
    if "nc" not in _CACHE:
        _CACHE["nc"] = _build()
    nc = _CACHE["nc"]
    in_maps = _prep_inputs(x, clusters, clusters2)
    res = run_bass_kernel_spmd(nc, in_maps, list(range(NCORES)), trace=_TRACE)
    _LAST_RESULT = res
    # per-core y is [K, BPC, D] -> [BPC, K, D]
    y = np.stack([np.asarray(res.results[i]["y"]) for i in range(NCORES)])
    y = y.astype(np.float32) * 4.0
    return np.ascontiguousarray(y.transpose(0, 2, 1, 3)).reshape(B_FULL, K * D)
